# revision 1
# baseline (speedup 1.0000x reference)
"""Trainium2 Bass kernel for the adapted C-Mamba block.

Self-contained: takes FULL inputs as numpy arrays, shards the flattened
batch*num_nodes sequence axis across 8 NeuronCores (data parallel), runs a
feature-major Bass/Tile kernel per core, and gathers the full output.

Per-core dataflow (S=128 sequences, T=64 steps, R=S*T=8192 rows):
  - RMSNorm scale per row (norm_w folded into in_proj weights on host)
  - PE transpose to feature-major xT (dm=128, R)
  - in_proj / causal depthwise conv (shifted fused mul-adds) / x_proj /
    dt_proj+softplus, all feature-major
  - SSM scan: partitions = (d-group-of-8, n=16); per d-tile the small
    selector matmuls replicate delta/w across n and B across d; ScalarE
    computes dA=exp(A*delta) (A is a per-partition scalar); VectorE's
    hardware prefix-scan (tensor_tensor_scan) runs the recurrence along
    the free axis, chained across sequences with dA=0 resets at t=0
  - y = sum_n h*C via a block-ones matmul, gating, out_proj, PE transpose
    back to row-major, residual add, store.
"""
import os
import sys

import numpy as np

for _p in ("/opt/trn_rl_repo",):
    if _p not in sys.path and os.path.isdir(_p):
        sys.path.append(_p)

import concourse.bacc as bacc  # noqa: E402
import concourse.bass as bass  # noqa: E402
import concourse.tile as tile  # noqa: E402
from concourse import mybir  # noqa: E402
from concourse.bass_utils import run_bass_kernel_spmd  # noqa: E402

F32 = mybir.dt.float32
BF16 = mybir.dt.bfloat16
AF = mybir.ActivationFunctionType
OP = mybir.AluOpType

B, L, N, DM = 8, 64, 128, 128
DS, DCONV, DFF, DTR = 16, 4, 256, 16
EPS = 1e-5
NCORES = 8
SEQ_PER_CORE = (B * N) // NCORES      # 128
R = SEQ_PER_CORE * L                  # 8192 rows per core
T = L                                 # 64
NCH = 8                               # chunks per core
ST = R // NCH                         # 1024 cols per chunk (16 seqs)
RT = ST // 128                        # 8 row-tiles per chunk
NSUB = ST // 512                      # 512-col sub-tiles per chunk


def _seqslice(ap2d):
    """view a (128, ST) AP as (128, seqs, T)"""
    return ap2d.rearrange("p (s t) -> p s t", t=T)


def build_nc():
    nc = bacc.Bacc()
    x_in = nc.declare_dram_parameter("x", [R, DM], F32, isOutput=False)
    w1t_in = nc.declare_dram_parameter("w1t", [128, 512], BF16, isOutput=False)
    b1_in = nc.declare_dram_parameter("b1", [128, 4], F32, isOutput=False)
    cw_in = nc.declare_dram_parameter("cw", [128, 8], F32, isOutput=False)
    cb_in = nc.declare_dram_parameter("cb", [128, 2], F32, isOutput=False)
    xpt_in = nc.declare_dram_parameter("xpt", [128, 96], BF16, isOutput=False)
    dtt_in = nc.declare_dram_parameter("dtt", [16, 256], BF16, isOutput=False)
    dtb_in = nc.declare_dram_parameter("dtb", [128, 2], F32, isOutput=False)
    acol_in = nc.declare_dram_parameter("acol", [128, 32], F32, isOutput=False)
    sel16_in = nc.declare_dram_parameter("sel16", [48, 256], BF16, isOutput=False)
    sel8_in = nc.declare_dram_parameter("sel8", [128, 2048], BF16, isOutput=False)
    lsum_in = nc.declare_dram_parameter("lsum", [128, 2048], BF16, isOutput=False)
    ident_in = nc.declare_dram_parameter("ident", [128, 128], F32, isOutput=False)
    identb_in = nc.declare_dram_parameter("identb", [128, 128], BF16, isOutput=False)
    dvec_in = nc.declare_dram_parameter("dvec", [128, 2], F32, isOutput=False)
    owt_in = nc.declare_dram_parameter("owt", [128, 256], BF16, isOutput=False)
    opb_in = nc.declare_dram_parameter("opb", [128, 1], F32, isOutput=False)
    misc_in = nc.declare_dram_parameter("misc", [128, 3], F32, isOutput=False)
    out_dram = nc.declare_dram_parameter("out", [R, DM], F32, isOutput=True)

    x_view = x_in[:].rearrange("(n p) d -> n p d", p=128)
    x_pnd = x_in[:].rearrange("(n p) d -> p n d", p=128)
    out_view = out_dram[:].rearrange("(n p) d -> n p d", p=128)

    with tile.TileContext(nc) as tc:
        with (
            tc.tile_pool(name="const", bufs=1) as cpool,
            tc.tile_pool(name="xrows", bufs=3) as xrows_pool,
            tc.tile_pool(name="xn", bufs=2) as xn_pool,
            tc.tile_pool(name="small", bufs=4) as small,
            tc.tile_pool(name="xt", bufs=3) as xt_pool,
            tc.tile_pool(name="big", bufs=2) as big,
            tc.tile_pool(name="big1", bufs=2) as big1,
            tc.tile_pool(name="dbcp", bufs=2) as dbc_pool,
            tc.tile_pool(name="bc", bufs=2) as bc_pool,
            tc.tile_pool(name="scan", bufs=3) as scan_pool,
            tc.tile_pool(name="epool", bufs=2) as e_pool,
            tc.tile_pool(name="orow", bufs=6) as orow_pool,
            tc.tile_pool(name="mm", bufs=2, space="PSUM") as mm_psum,
            tc.tile_pool(name="rep", bufs=2, space="PSUM") as rep_psum,
            tc.tile_pool(name="ypsum", bufs=1, space="PSUM") as y_psum_pool,
        ):
            def cload(name, dram, shape, dt=F32):
                t = cpool.tile(shape, dt, tag=name)
                nc.sync.dma_start(t[:], dram[:])
                return t

            c_w1t = cload("w1t", w1t_in, [128, 512], BF16)
            c_b1 = cload("b1", b1_in, [128, 4])
            c_cw = cload("cw", cw_in, [128, 8])
            c_cb = cload("cb", cb_in, [128, 2])
            c_xpt = cload("xpt", xpt_in, [128, 96], BF16)
            c_dtt = cload("dtt", dtt_in, [16, 256], BF16)
            c_dtb = cload("dtb", dtb_in, [128, 2])
            c_acol = cload("acol", acol_in, [128, 32])
            c_sel16 = cload("sel16", sel16_in, [48, 256], BF16)
            c_sel8 = cload("sel8", sel8_in, [128, 2048], BF16)
            c_lsum = cpool.tile([128, 2048], BF16, tag="lsum")
            nc.sync.dma_start(c_lsum[:], lsum_in[:])
            c_ident = cload("ident", ident_in, [128, 128])
            c_identb = cload("identb", identb_in, [128, 128], BF16)
            c_dvec = cload("dvec", dvec_in, [128, 2])
            c_owt = cload("owt", owt_in, [128, 256], BF16)
            c_opb = cload("opb", opb_in, [128, 1])
            c_misc = cload("misc", misc_in, [128, 3])
            nc.const_aps.aps[(F32, 0.0)] = c_misc[:, 0:1]
            nc.const_aps.aps[(F32, EPS)] = c_misc[:, 1:2]
            nc.const_aps.aps[(F32, 1.0)] = c_misc[:, 2:3]

            for c in range(NCH):
                # ---- load + RMS norm + transpose ----
                xr = xrows_pool.tile([128, RT, 128], F32, tag="xr")
                nc.sync.dma_start(
                    xr[:], x_pnd[:, c * RT:(c + 1) * RT, :])
                ssq = small.tile([128, RT], F32, tag="ssq")
                for rt in range(RT):
                    scr = small.tile([128, 128], F32, tag="sqscr")
                    nc.scalar.activation(
                        scr[:], xr[:, rt, :], AF.Square,
                        accum_out=ssq[:, rt:rt + 1])
                vtmp = small.tile([128, RT], F32, tag="vtmp")
                nc.scalar.activation(vtmp[:], ssq[:], AF.Identity,
                                     bias=EPS, scale=1.0 / DM)
                rinv = small.tile([128, RT], F32, tag="rinv")
                nc.vector.reciprocal(rinv[:], vtmp[:])
                rln = small.tile([128, RT], F32, tag="rln")
                nc.scalar.activation(rln[:], rinv[:], AF.Ln)
                rstd = small.tile([128, RT], F32, tag="rstd")
                nc.scalar.activation(rstd[:], rln[:], AF.Exp, scale=0.5)
                xn = xn_pool.tile([128, RT, 128], F32, tag="xn")
                for rt in range(RT):
                    nc.vector.tensor_scalar_mul(
                        xn[:, rt, :], xr[:, rt, :], rstd[:, rt:rt + 1])
                xT = xt_pool.tile([128, ST], BF16, tag="xT")
                for rt in range(RT):
                    pt = mm_psum.tile([128, 128], F32, tag="mm")
                    nc.tensor.transpose(pt[:], xn[:, rt, :], c_ident[:])
                    nc.scalar.copy(xT[:, rt * 128:(rt + 1) * 128], pt[:])

                # ---- in_proj (features on partitions) ----
                xinT = big1.tile([128, 2, ST], BF16, tag="xinT")
                zsT = big.tile([128, 2, ST], BF16, tag="zsT")
                for f in range(4):
                    for sub in range(NSUB):
                        sl = slice(sub * 512, (sub + 1) * 512)
                        ps = mm_psum.tile([128, 512], F32, tag="mm")
                        nc.tensor.matmul(
                            ps[:], c_w1t[:, f * 128:(f + 1) * 128],
                            xT[:, sl], start=True, stop=True)
                        if f < 2:
                            nc.scalar.activation(
                                xinT[:, f, sl], ps[:], AF.Identity,
                                bias=c_b1[:, f:f + 1])
                        else:
                            nc.scalar.activation(
                                zsT[:, f - 2, sl], ps[:], AF.Silu,
                                bias=c_b1[:, f:f + 1])

                # ---- causal depthwise conv + silu ----
                xcp = big1.tile([128, 2, ST], BF16, tag="xcp")
                xcT = big.tile([128, 2, ST], BF16, tag="xcT")
                for h in range(2):
                    xin3 = _seqslice(xinT[:, h, :])
                    xc3 = _seqslice(xcp[:, h, :])
                    nc.vector.tensor_scalar_mul(
                        xc3[:, :, :], xin3[:, :, :],
                        c_cw[:, h * 4 + 3:h * 4 + 4])
                    for shift in (1, 2, 3):
                        j = 3 - shift
                        nc.vector.scalar_tensor_tensor(
                            xc3[:, :, shift:], xin3[:, :, :T - shift],
                            c_cw[:, h * 4 + j:h * 4 + j + 1],
                            xc3[:, :, shift:], op0=OP.mult, op1=OP.add)
                    nc.scalar.activation(
                        xcT[:, h, :], xcp[:, h, :], AF.Silu,
                        bias=c_cb[:, h:h + 1])

                # ---- x_proj -> (dr, B, C) ----
                dbc = dbc_pool.tile([48, ST], BF16, tag="dbc")
                for sub in range(NSUB):
                    sl = slice(sub * 512, (sub + 1) * 512)
                    ps48 = mm_psum.tile([48, 512], F32, tag="mm")
                    nc.tensor.matmul(ps48[:], c_xpt[:, 0:48], xcT[:, 0, sl],
                                     start=True, stop=False)
                    nc.tensor.matmul(ps48[:], c_xpt[:, 48:96], xcT[:, 1, sl],
                                     start=False, stop=True)
                    nc.scalar.copy(dbc[:, sl], ps48[:])

                # ---- dt_proj + softplus ----
                dw = big.tile([128, 2, 2, ST], BF16, tag="dw")
                for h in range(2):
                    for sub in range(NSUB):
                        sl = slice(sub * 512, (sub + 1) * 512)
                        ps = mm_psum.tile([128, 512], F32, tag="mm")
                        nc.tensor.matmul(
                            ps[:], c_dtt[:, h * 128:(h + 1) * 128],
                            dbc[0:16, sl], start=True, stop=True)
                        spt = small.tile([128, 512], F32, tag="sptmp")
                        nc.scalar.activation(spt[:], ps[:], AF.Exp,
                                             bias=c_dtb[:, h:h + 1])
                        nc.scalar.activation(dw[:, h, 0, sl], spt[:],
                                             AF.Ln, bias=1.0)

                # ---- w = delta * xc ----
                for h in range(2):
                    nc.vector.tensor_mul(dw[:, h, 1, :], dw[:, h, 0, :],
                                         xcT[:, h, :])

                # ---- B_rep / C_rep (replicate across d within partition tile) ----
                reps = []
                for name, si in (("Brep", 0), ("Crep", 1)):
                    psr = rep_psum.tile([128, ST], F32, tag="rep")
                    for sub in range(NSUB):
                        sl = slice(sub * 512, (sub + 1) * 512)
                        nc.tensor.matmul(
                            psr[:, sl], c_sel16[:, si * 128:(si + 1) * 128],
                            dbc[:, sl], start=True, stop=True)
                    sb = bc_pool.tile([128, ST], BF16, tag=name)
                    nc.scalar.copy(sb[:], psr[:])
                    reps.append(sb)
                Brep, Crep = reps

                # ---- scan over d-tiles ----
                y2 = big1.tile([128, 2, ST], BF16, tag="y2")
                for h in range(2):
                    yps = y_psum_pool.tile([128, ST], F32, tag="y")
                    for kk in range(16):
                        k = h * 16 + kk
                        po = 8 * kk
                        selk = c_sel8[:, kk * 128:(kk + 1) * 128]
                        dpsum = rep_psum.tile([128, ST], F32, tag="rep")
                        for sub in range(NSUB):
                            sl = slice(sub * 512, (sub + 1) * 512)
                            nc.tensor.matmul(
                                dpsum[:, sl], selk, dw[:, h, 0, sl],
                                start=True, stop=True)
                        dA = scan_pool.tile([128, ST], BF16, tag="dA")
                        nc.scalar.activation(dA[:], dpsum[:], AF.Exp,
                                             scale=c_acol[:, k:k + 1])
                        dA3 = _seqslice(dA[:])
                        nc.vector.memset(dA3[:, :, 0:1], 0.0)
                        wpsum = rep_psum.tile([128, ST], F32, tag="rep")
                        for sub in range(NSUB):
                            sl = slice(sub * 512, (sub + 1) * 512)
                            nc.tensor.matmul(
                                wpsum[:, sl], selk, dw[:, h, 1, sl],
                                start=True, stop=True)
                        wrs = scan_pool.tile([128, ST], BF16, tag="wrs")
                        nc.scalar.copy(wrs[:], wpsum[:])
                        g = scan_pool.tile([128, ST], BF16, tag="g")
                        nc.vector.tensor_mul(g[:], wrs[:], Brep[:])
                        hsb = scan_pool.tile([128, ST], BF16, tag="h")
                        nc.vector.tensor_tensor_scan(
                            hsb[:], dA[:], g[:], 0.0,
                            op0=OP.mult, op1=OP.add)
                        msb = scan_pool.tile([128, ST], BF16, tag="m")
                        nc.vector.tensor_mul(msb[:], hsb[:], Crep[:])
                        for sub in range(NSUB):
                            sl = slice(sub * 512, (sub + 1) * 512)
                            nc.tensor.matmul(
                                yps[:, sl],
                                c_lsum[:, kk * 128:(kk + 1) * 128],
                                msb[:, sl],
                                start=(kk == 0), stop=(kk == 15))
                    esb = e_pool.tile([128, ST], BF16, tag="e")
                    nc.vector.scalar_tensor_tensor(
                        esb[:], xcT[:, h, :], c_dvec[:, h:h + 1], yps[:],
                        op0=OP.mult, op1=OP.add)
                    nc.vector.tensor_mul(y2[:, h, :], esb[:], zsT[:, h, :])

                # ---- out_proj + transpose back + residual ----
                oT = xt_pool.tile([128, ST], BF16, tag="oT")
                for sub in range(NSUB):
                    sl = slice(sub * 512, (sub + 1) * 512)
                    ps = mm_psum.tile([128, 512], F32, tag="mm")
                    nc.tensor.matmul(ps[:], c_owt[:, 0:128], y2[:, 0, sl],
                                     start=True, stop=False)
                    nc.tensor.matmul(ps[:], c_owt[:, 128:256], y2[:, 1, sl],
                                     start=False, stop=True)
                    nc.scalar.activation(oT[:, sl], ps[:], AF.Identity,
                                         bias=c_opb[:, 0:1])
                for rt in range(RT):
                    pt = mm_psum.tile([128, 128], BF16, tag="mm")
                    nc.tensor.transpose(pt[:], oT[:, rt * 128:(rt + 1) * 128],
                                        c_identb[:])
                    orow = orow_pool.tile([128, 128], F32, tag="orow")
                    nc.vector.tensor_add(orow[:], pt[:], xr[:, rt, :])
                    nc.sync.dma_start(out_view[c * RT + rt], orow[:])
    nc.finalize()
    return nc


def host_prep(inp):
    import ml_dtypes
    f = lambda a: np.ascontiguousarray(np.asarray(a, np.float32))
    fb = lambda a: np.ascontiguousarray(
        np.asarray(a, np.float32).astype(ml_dtypes.bfloat16))
    wd = {}
    wd["w1t"] = fb((np.asarray(inp["in_proj_w"], np.float32)
                   * np.asarray(inp["norm_w"], np.float32)[None, :]).T)
    wd["b1"] = f(np.asarray(inp["in_proj_b"], np.float32).reshape(4, 128).T)
    cwf = np.asarray(inp["conv_w"], np.float32)[:, 0, :]        # (256,4)
    wd["cw"] = f(np.concatenate([cwf[:128], cwf[128:]], axis=1))  # (128,8)
    wd["cb"] = f(np.asarray(inp["conv_b"], np.float32).reshape(2, 128).T)
    xpt = np.asarray(inp["x_proj_w"], np.float32).T             # (256,48)
    wd["xpt"] = fb(np.concatenate([xpt[:128], xpt[128:]], axis=1))  # (128,96)
    wd["dtt"] = fb(np.asarray(inp["dt_proj_w"], np.float32).T)   # (16,256)
    wd["dtb"] = f(np.asarray(inp["dt_proj_b"], np.float32).reshape(2, 128).T)
    A = -np.exp(np.asarray(inp["A_log"], np.float32))           # (256,16)
    wd["acol"] = f(A.reshape(32, 8 * 16).T.reshape(128, 32))
    p = np.arange(128)
    sel16 = np.zeros((48, 2, 128), np.float32)
    for si, off in ((0, 16), (1, 32)):
        sel16[:, si, :] = (np.arange(48)[:, None] == off + p[None, :] % 16)
    wd["sel16"] = fb(sel16.reshape(48, 256))
    sel8 = np.zeros((128, 16, 128), np.float32)
    for kk in range(16):
        sel8[:, kk, :] = (np.arange(128)[:, None] == 8 * kk + p[None, :] // 16)
    wd["sel8"] = fb(sel8.reshape(128, 2048))
    lsum = np.zeros((128, 16, 128), np.float32)
    for kk in range(16):
        for j in range(8):
            lsum[:, kk, 8 * kk + j] = (p // 16 == j)
    wd["lsum"] = np.ascontiguousarray(
        lsum.reshape(128, 2048).astype(ml_dtypes.bfloat16))
    wd["ident"] = f(np.eye(128))
    wd["identb"] = fb(np.eye(128))
    wd["dvec"] = f(np.asarray(inp["D"], np.float32).reshape(2, 128).T)
    wd["owt"] = fb(np.asarray(inp["out_proj_w"], np.float32).T
                  .reshape(2, 128, 128).transpose(1, 0, 2).reshape(128, 256))
    wd["opb"] = f(np.asarray(inp["out_proj_b"], np.float32).reshape(128, 1))
    misc = np.zeros((128, 3), np.float32)
    misc[:, 1] = EPS
    misc[:, 2] = 1.0
    wd["misc"] = misc
    return wd


_CACHE = {}


def kernel(**inputs) -> np.ndarray:
    if "nc" not in _CACHE:
        _CACHE["nc"] = build_nc()
    nc = _CACHE["nc"]
    wd = host_prep(inputs)
    xf = np.ascontiguousarray(
        np.asarray(inputs["x"], np.float32).reshape(B * L * N, DM))
    in_maps = []
    for c in range(NCORES):
        m = dict(wd)
        m["x"] = np.ascontiguousarray(xf[c * R:(c + 1) * R])
        in_maps.append(m)
    res = run_bass_kernel_spmd(nc, in_maps, list(range(NCORES)))
    outs = [np.asarray(res.results[c]["out"]) for c in range(NCORES)]
    return np.concatenate(outs, 0).reshape(B, L, N, DM).astype(np.float32)


if __name__ == "__main__":
    nc = build_nc()
    print("built ok")

